# revision 1
# baseline (speedup 1.0000x reference)
"""GCNSimple Trainium2 kernel: 8-core data-parallel (graphs sharded),
gather-only message passing (dma_gather + in-SBUF pow2 tree reduction; no
scatter-add). 3 device launches: A (embed), B (layer1 + z), C (layer2 + pool).
Host does only index/layout preprocessing and layout shuffles between launches.
"""
import sys
sys.path.insert(0, "/opt/trn_rl_repo")
import numpy as np

import concourse.bacc as bacc
import concourse.mybir as mybir
from concourse import ap_utils
from concourse.bass import MemorySpace, AP as _APc, IndirectOffsetOnAxis
from concourse.tile import TileContext, add_dep_helper
from concourse._compat import exact_div
from concourse.bass_utils import run_bass_kernel_spmd

P = 128
NCORES = 8
N_NODES = 200_000
N_EDGES = 6_400_000
N_GRAPHS = 512
F_IN = 92
W = 10
GPC = N_GRAPHS // NCORES
CP = 64                 # slots per partition per gather chunk
NI = P * CP             # 8192 slots per chunk
STEP = 64               # table row stride (f32) = 256B
GRANULES = (32, 16, 8, 4, 2)
NQ = 4
FP = np.float32


# ---------------------------------------------------------------- raw gather
def dma_gather_raw(gp, out_ap, in_ap, idxs_ap, num_idxs, elem_size, elem_step,
                   queue_num=0, single_packet=False):
    gp._assert_queue_num(queue_num)
    assert idxs_ap.dtype == mybir.dt.int16
    assert in_ap.dtype == out_ap.dtype
    assert in_ap.space == MemorySpace.DRAM
    assert ap_utils.ap_is_contiguous(in_ap.ap[1:])
    assert ap_utils.ap_is_contiguous(out_ap.ap[1:])
    assert ap_utils.ap_is_contiguous(idxs_ap.ap[1:])
    assert in_ap.ap[-1][1] == out_ap.ap[-1][1] == elem_size
    assert out_ap.ap[0][1] * out_ap.ap[1][1] == (num_idxs + 127) // 128 * 128
    assert in_ap.ap[0][0] == elem_step
    stride_bytes_256 = exact_div(elem_step * mybir.dt.size(in_ap.dtype), 256)
    return gp.add_instruction(
        mybir.InstDMAGatherAnt(
            name=gp.bass.get_next_instruction_name(),
            ins=[*gp.lower_ap_dma(in_ap, for_custom_bir_dma=True),
                 gp.lower_ap(idxs_ap),
                 gp.lower_val_access(gp.to_reg(num_idxs))],
            outs=[gp.lower_ap(out_ap)],
            transpose=False, num_idxs=num_idxs, elem_size=elem_size,
            stride_bytes_256=stride_bytes_256, gen_mode=0,
            single_packet=single_packet, queue_num=queue_num,
            sbuf_tokens_per_rank=0, sbuf_free_dim_per_rank=0,
            sbuf_free_dim_pad_per_rank=0, sbuf_byte_offset=0,
        ))


def wrap_idx16(logical_idx, num_idxs):
    w = np.zeros((16, num_idxs // 16), np.int16)
    ar = np.arange(num_idxs)
    w[ar % 16, ar // 16] = logical_idx.astype(np.int16)
    return np.tile(w, (8, 1))


# ---------------------------------------------------------------- host plan
def build_plan(src, dst, graph_ids, r_in):
    src = np.asarray(src, np.int64)
    dst = np.asarray(dst, np.int64)
    graph_ids = np.asarray(graph_ids, np.int64)

    gstart = np.searchsorted(graph_ids, np.arange(N_GRAPHS + 1))
    gsz = np.diff(gstart)
    core_n1 = gstart[np.arange(NCORES) * GPC + GPC]
    nodecore = np.searchsorted(core_n1, np.arange(N_NODES), side="right")
    g_of_n = np.repeat(np.arange(N_GRAPHS), gsz)

    ncols_g = np.maximum((gsz + P - 1) // P, 1)
    cstart_g = np.zeros(N_GRAPHS, np.int64)
    COLS_c = np.zeros(NCORES, np.int64)
    for c in range(NCORES):
        cs = np.cumsum(ncols_g[c * GPC:(c + 1) * GPC])
        cstart_g[c * GPC + 1:(c + 1) * GPC] = cs[:-1]
        COLS_c[c] = cs[-1]
    COLS = int(COLS_c.max())
    NPAD = P * COLS
    NTAB = NPAD + 1
    assert NTAB < 32760
    lp = cstart_g[g_of_n] * P + (np.arange(N_NODES) - gstart[g_of_n])

    outdeg = np.bincount(src, minlength=N_NODES).astype(np.int32)
    indeg = np.bincount(dst, minlength=N_NODES).astype(np.int32)

    def nodearr(vals):
        out = np.zeros((NCORES, P, COLS), vals.dtype)
        out[nodecore, lp % P, lp // P] = vals
        return out

    # ---- per-core run lists (L1) and edge lists (L3)
    ecore = nodecore[dst]
    erange = nodecore[src]
    cores = []
    cellcnt = np.zeros((NCORES, NCORES, len(GRANULES)), np.int64)  # runs per cell
    for c in range(NCORES):
        em = np.nonzero(ecore == c)[0]
        er, ed = erange[em], dst[em]
        order = np.lexsort((src[em], ed, er))
        em, er, ed = em[order], er[order], ed[order]
        chg = np.empty(len(em), bool)
        chg[0] = True
        chg[1:] = (er[1:] != er[:-1]) | (ed[1:] != ed[:-1])
        rstart = np.nonzero(chg)[0]
        rlen = np.diff(np.append(rstart, len(em)))
        assert rlen.max() <= GRANULES[0]
        gran = np.zeros(len(rlen), np.int64)
        gi = np.zeros(len(rlen), np.int64)
        for k, g in enumerate(GRANULES):
            m = rlen <= g
            gran[m] = g
            gi[m] = k
        rrange = er[rstart]
        for r in range(NCORES):
            for k in range(len(GRANULES)):
                cellcnt[c, r, k] = np.count_nonzero((rrange == r) & (gi == k))
        cores.append(dict(em=em, er=er, ed=ed, rstart=rstart, rlen=rlen,
                          gran=gran, gi=gi, rrange=rrange))

    # ---- uniform chunk schedule (shared program): cells (range, granule)
    sched = []          # list of (range, granule, n_run_slots_in_chunk=NI//g)
    cell_chunk0 = np.zeros((NCORES, len(GRANULES)), np.int64)
    cell_nch = np.zeros((NCORES, len(GRANULES)), np.int64)
    for r in range(NCORES):
        for k, g in enumerate(GRANULES):
            mx = int(cellcnt[:, r, k].max())
            rpc = P * (CP // g)
            if g == 2:
                mx += 1  # spare run slot -> guaranteed zero row per range
            nch = (mx + rpc - 1) // rpc
            cell_chunk0[r, k] = len(sched)
            cell_nch[r, k] = nch
            sched += [(r, g)] * nch
    NCH2 = len(sched)
    sched_r = np.array([s[0] for s in sched])
    sched_g = np.array([s[1] for s in sched])

    # pbuf layout (uniform): per range regions; rows per chunk = P*(CP//g)
    chunkrow0 = np.zeros(NCH2, np.int64)
    rangerows = np.zeros(NCORES, np.int64)
    for i in range(NCH2):
        r, g = sched[i]
        chunkrow0[i] = rangerows[r]
        rangerows[r] += P * (CP // g)
    PBR = int(rangerows.max())
    assert PBR < 32760
    rbase = np.zeros(NCORES, np.int64)  # pbuf row base per range
    for r in range(1, NCORES):
        rbase[r] = rbase[r - 1] + PBR

    RS = [(0, min(96, COLS))]
    if COLS > 96:
        RS.append((96, min(192, COLS)))
    if COLS > 192:
        RS.append((192, COLS))

    # ---- per-core packing into the uniform schedule
    gidx2 = np.zeros((NCORES, NCH2 * P, NI // 16), np.int16)
    rP2 = np.zeros((NCORES, NCH2 * P, CP * 3), FP)
    rnd = [np.zeros((NCORES, NCORES * P, (b - a) * P // 16), np.int16)
           for a, b in RS]
    l2slot_of_edge = np.full(N_EDGES, -1, np.int64)
    for c in range(NCORES):
        cr = cores[c]
        gl = np.full((NCH2 * NI,), NTAB - 1, np.int32)
        rme = np.zeros((NCH2 * NI, 3), FP)
        rowrel = np.zeros(len(cr["rlen"]), np.int64)
        for r in range(NCORES):
            for k, g in enumerate(GRANULES):
                runs = np.nonzero((cr["rrange"] == r) & (cr["gi"] == k))[0]
                rpc = P * (CP // g)
                rpp = CP // g
                w_ = np.arange(len(runs))
                ch = cell_chunk0[r, k] + w_ // rpc
                wi = w_ % rpc
                p_ = wi // rpp
                cb = wi % rpp
                s0 = ch * NI + (cb * g) * P + p_
                rowrel[runs] = chunkrow0[ch] + p_ * rpp + cb
                if len(runs) == 0:
                    continue
                lens = cr["rlen"][runs]
                starts = cr["rstart"][runs]
                tt = int(lens.sum())
                off = np.zeros(len(runs), np.int64)
                off[1:] = np.cumsum(lens)[:-1]
                jj = np.arange(tt) - off.repeat(lens)
                epos = starts.repeat(lens) + jj
                sl = s0.repeat(lens) + jj * P
                ee = cr["em"][epos]
                gl[sl] = lp[src[ee]]
                rme[sl] = r_in[ee]
                l2slot_of_edge[ee] = sl
        for i in range(NCH2):
            s = slice(i * NI, (i + 1) * NI)
            gidx2[c, i * P:(i + 1) * P] = wrap_idx16(gl[s], NI)
            rP2[c, i * P:(i + 1) * P] = (
                rme[s].reshape(CP, P, 3).transpose(1, 0, 2).reshape(P, CP * 3))
        # rounds: per range, per node col-range; zero target = last row of
        # the granule-2 cell region (spare run slot guaranteed by mx+1)
        k2 = len(GRANULES) - 1
        zr = (chunkrow0[cell_chunk0[:, k2] + cell_nch[:, k2] - 1]
              + P * (CP // 2) - 1)
        for r in range(NCORES):
            arr = np.full((NPAD,), rbase[r] + zr[r], np.int64)  # a zero row
            m = cr["rrange"] == r
            dl = lp[cr["ed"][cr["rstart"][m]]]
            arr[dl] = rbase[r] + rowrel[m]
            for si, (a, b) in enumerate(RS):
                rnd[si][c, r * P:(r + 1) * P] = wrap_idx16(
                    arr[a * P:b * P] - rbase[r], (b - a) * P)

    # ---- L3 layout: per range sections, (r, g) whole-column segments,
    # range sections padded to CP-column boundaries (single-range chunks)
    cnt3 = np.zeros((NCORES, NCORES, GPC), np.int64)
    for c in range(NCORES):
        cr = cores[c]
        key = cr["er"] * GPC + (g_of_n[cr["ed"]] - c * GPC)
        cnt3[c] = np.bincount(key, minlength=NCORES * GPC).reshape(NCORES, GPC)
    padcols = (cnt3 + P - 1) // P  # cols per (c, r, g)
    cols_rg = padcols.max(axis=0)  # uniform cols per (r, g)
    # range section cols padded to CP multiple
    seccols = np.zeros(NCORES, np.int64)
    segcol0 = np.zeros((NCORES, GPC), np.int64)
    secbase = np.zeros(NCORES, np.int64)
    tot = 0
    for r in range(NCORES):
        secbase[r] = tot
        cc = np.cumsum(np.concatenate([[0], cols_rg[r]]))
        segcol0[r] = secbase[r] + cc[:-1]
        sc = int((cc[-1] + CP - 1) // CP * CP)
        seccols[r] = sc
        tot += sc
    NCOL3 = tot
    NCH3 = NCOL3 // CP
    sched3_r = np.zeros(NCH3, np.int64)
    for r in range(NCORES):
        sched3_r[secbase[r] // CP:(secbase[r] + seccols[r]) // CP] = r

    gidx3 = np.zeros((NCORES, NCH3 * P, NI // 16), np.int16)
    hibnd = np.zeros((NCORES, 4, P), np.int32)
    lobnd = np.zeros((NCORES, 4, P), np.int32)
    l3slot_of_edge = np.full(N_EDGES, -1, np.int64)
    for c in range(NCORES):
        cr = cores[c]
        gl3 = np.full((NCH3 * NI,), NTAB - 1, np.int32)
        key = cr["er"] * GPC + (g_of_n[cr["ed"]] - c * GPC)
        o3 = np.argsort(key, kind="stable")
        e3 = cr["em"][o3]
        k3 = key[o3]
        seg_first = np.searchsorted(k3, np.arange(NCORES * GPC))
        off = np.arange(len(e3)) - seg_first[k3]
        slot3 = segcol0.reshape(-1)[k3] * P + off
        gl3[slot3] = lp[src[e3]]
        l3slot_of_edge[e3] = slot3
        for i in range(NCH3):
            s = slice(i * NI, (i + 1) * NI)
            gidx3[c, i * P:(i + 1) * P] = wrap_idx16(gl3[s], NI)
        for gl_ in range(GPC):
            for r in range(NCORES):
                j = gl_ * NCORES + r
                c0 = segcol0[r, gl_]
                hibnd[c, j // P, j % P] = c0 + cols_rg[r, gl_]
                lobnd[c, j // P, j % P] = c0
    grouphot = np.zeros((P, 16), FP)
    grouphot[np.arange(P), np.arange(P) // 8] = 1.0

    cnt_pg = np.zeros((NCORES, 4, 16), np.int32)
    for c in range(NCORES):
        for gl_ in range(GPC):
            cnt_pg[c, gl_ // 16, gl_ % 16] = gsz[c * GPC + gl_]

    return dict(
        COLS=COLS, NPAD=NPAD, NTAB=NTAB, NCH2=NCH2, NCH3=NCH3, PBR=PBR,
        sched_r=sched_r, sched_g=sched_g, chunkrow0=chunkrow0, rbase=rbase,
        RS=RS, sched3_r=sched3_r,
        gidx2=gidx2, rP2=rP2, rnd=rnd, gidx3=gidx3,
        hibnd=hibnd, lobnd=lobnd, grouphot=grouphot, cnt_pg=cnt_pg,
        outdeg_l=nodearr(outdeg), indeg_l=nodearr(indeg),
        lp=lp, nodecore=nodecore, g_of_n=g_of_n, gsz=gsz,
        l2slot=l2slot_of_edge, l3slot=l3slot_of_edge,
        ecore=ecore, src=src, dst=dst,
    )


# ---------------------------------------------------------------- device APs
def _bc(t_ap, dims):
    """Broadcast AP from a tile slice: dims = list of [step, count]."""
    return _APc(t_ap.tensor, t_ap.offset, [list(d) for d in dims])


def _mk_nc():
    return bacc.Bacc("TRN2", target_bir_lowering=False, debug=False,
                     num_swdge_queues=NQ)


# ---------------------------------------------------------------- launch A
def build_A(COLS, reps=0):
    nc = _mk_nc()
    f32, i32 = mybir.dt.float32, mybir.dt.int32
    AT = nc.dram_tensor("AT", [F_IN, P * COLS], f32, kind="ExternalInput")
    WE = nc.dram_tensor("WE", [F_IN, W], f32, kind="ExternalInput")
    BE = nc.dram_tensor("BE", [P, W], f32, kind="ExternalInput")
    OD = nc.dram_tensor("OD", [P, COLS], i32, kind="ExternalInput")
    HS = nc.dram_tensor("HS", [P, COLS * W], f32, kind="ExternalOutput")
    with TileContext(nc) as tc, \
         tc.tile_pool(name="sb", bufs=2) as pool, \
         tc.tile_pool(name="ps", bufs=8, space="PSUM") as psp:
        at = pool.tile([F_IN, P * COLS], f32, bufs=1)
        nc.sync.dma_start(out=at[:], in_=AT[:])
        we = pool.tile([F_IN, W], f32, bufs=1)
        nc.sync.dma_start(out=we[:], in_=WE[:])
        be = pool.tile([P, W], f32, bufs=1)
        nc.sync.dma_start(out=be[:], in_=BE[:])
        od = pool.tile([P, COLS], i32, bufs=1)
        nc.sync.dma_start(out=od[:], in_=OD[:])
        hs = pool.tile([P, COLS * W], f32, bufs=1)
        import contextlib
        loop_cm = tc.For_i(0, reps, 1) if reps else contextlib.nullcontext()
        loop_cm.__enter__()
        for c in range(COLS):
            ps = psp.tile([P, W], f32, tag="mm")
            nc.tensor.matmul(out=ps[:], lhsT=at[:, c * P:(c + 1) * P],
                             rhs=we[:], start=True, stop=True)
            nc.vector.tensor_tensor(
                out=hs[:, c * W:(c + 1) * W], in0=ps[:], in1=be[:],
                op=mybir.AluOpType.add)
        # inv_out = 1/sqrt(max(deg,1))
        odf = pool.tile([P, COLS], f32, bufs=1)
        nc.vector.tensor_copy(out=odf[:], in_=od[:])
        nc.vector.tensor_scalar(out=odf[:], in0=odf[:], scalar1=1.0,
                                scalar2=None, op0=mybir.AluOpType.max)
        sq = pool.tile([P, COLS], f32, bufs=1)
        nc.scalar.activation(out=sq[:], in_=odf[:],
                             func=mybir.ActivationFunctionType.Sqrt)
        inv = pool.tile([P, COLS], f32, bufs=1)
        nc.vector.reciprocal(out=inv[:], in_=sq[:])
        nc.vector.tensor_tensor(
            out=hs[:], in0=hs[:],
            in1=_bc(inv[:], [[COLS, P], [1, COLS], [0, W]]),
            op=mybir.AluOpType.mult)
        nc.sync.dma_start(out=HS[:], in_=hs[:])
        loop_cm.__exit__(None, None, None)
    nc.finalize()
    return nc


# ---------------------------------------------------------------- launch B
def build_B(plan, reps=0):
    COLS, NTAB, NCH2, PBR = plan["COLS"], plan["NTAB"], plan["NCH2"], plan["PBR"]
    RS = plan["RS"]
    sched_r, sched_g = plan["sched_r"], plan["sched_g"]
    chunkrow0 = plan["chunkrow0"]
    nc = _mk_nc()
    f32, i16, i32 = mybir.dt.float32, mybir.dt.int16, mybir.dt.int32
    HSP = nc.dram_tensor("HSP", [NCORES * NTAB, STEP], f32, kind="ExternalInput")
    GIX = nc.dram_tensor("GIX", [NCH2 * P, NI // 16], i16, kind="ExternalInput")
    RP = nc.dram_tensor("RP", [NCH2 * P, CP * 3], f32, kind="ExternalInput")
    RNDS = [nc.dram_tensor(f"RND{si}", [NCORES * P, (b - a) * P // 16], i16,
                           kind="ExternalInput") for si, (a, b) in enumerate(RS)]
    ID_ = nc.dram_tensor("IDG", [P, COLS], i32, kind="ExternalInput")
    OD_ = nc.dram_tensor("ODG", [P, COLS], i32, kind="ExternalInput")
    W1R = nc.dram_tensor("W1R", [P, W * W], f32, kind="ExternalInput")
    W2R = nc.dram_tensor("W2R", [P, W], f32, kind="ExternalInput")
    B1R = nc.dram_tensor("B1R", [P, W], f32, kind="ExternalInput")
    EWD = nc.dram_tensor("EWD", [NCH2 * P, CP], f32, kind="ExternalOutput")
    ZD = nc.dram_tensor("ZD", [P, COLS], f32, kind="ExternalOutput")
    IID = nc.dram_tensor("IID", [P, COLS], f32, kind="ExternalOutput")
    PBUF = nc.dram_tensor("PBUF", [NCORES * PBR, STEP], f32, kind="Internal")

    with TileContext(nc) as tc, \
         tc.tile_pool(name="sb", bufs=4) as pool:
        AL = mybir.AluOpType
        import contextlib
        loop_cm = tc.For_i(0, reps, 1) if reps else contextlib.nullcontext()
        loop_cm.__enter__()
        for i in range(NCH2):
            r, g = int(sched_r[i]), int(sched_g[i])
            rpp = CP // g
            gi = pool.tile([P, NI // 16], i16, tag="gi")
            nc.sync.dma_start(out=gi[:], in_=GIX[i * P:(i + 1) * P, :])
            gath = pool.tile([P, CP * W], f32, tag="gath")
            dma_gather_raw(
                nc.gpsimd,
                out_ap=gath[:].rearrange("p (c e) -> p c e", e=W),
                in_ap=HSP[r * NTAB:(r + 1) * NTAB, :W],
                idxs_ap=gi[:], num_idxs=NI, elem_size=W, elem_step=STEP,
                queue_num=i % NQ)
            rt = pool.tile([P, CP * 3], f32, tag="rt")
            nc.sync.dma_start(out=rt[:], in_=RP[i * P:(i + 1) * P, :])
            sq = pool.tile([P, CP * 3], f32, tag="sq")
            nc.vector.tensor_tensor(out=sq[:], in0=rt[:], in1=rt[:], op=AL.mult)
            s3 = sq[:].rearrange("p (c k) -> p c k", k=3)
            ssum = pool.tile([P, CP], f32, tag="ssum")
            nc.vector.tensor_tensor(out=ssum[:], in0=s3[:, :, 0], in1=s3[:, :, 1],
                                    op=AL.add)
            nc.vector.tensor_tensor(out=ssum[:], in0=ssum[:], in1=s3[:, :, 2],
                                    op=AL.add)
            ew = pool.tile([P, CP], f32, tag="ew")
            nc.scalar.activation(out=ew[:], in_=ssum[:],
                                 func=mybir.ActivationFunctionType.Exp,
                                 scale=-1.0)
            nc.sync.dma_start(out=EWD[i * P:(i + 1) * P, :], in_=ew[:])
            nc.vector.tensor_tensor(
                out=gath[:], in0=gath[:],
                in1=_bc(ew[:], [[CP, P], [1, CP], [0, W]]), op=AL.mult)
            # tree-reduce log2(g) levels
            cur = gath
            width = CP
            lv = int(np.log2(g))
            for l in range(lv):
                nxt = pool.tile([P, (width // 2) * W], f32, tag=f"tr{l}")
                cv = cur[:].rearrange("p (c e) -> p c e", e=W)
                nc.vector.tensor_tensor(
                    out=nxt[:].rearrange("p (c e) -> p c e", e=W),
                    in0=cv[:, 0::2, :], in1=cv[:, 1::2, :], op=AL.add)
                cur = nxt
                width //= 2
            # spill run sums (rpp per partition) padded to 64-f32 rows
            sp = pool.tile([P, rpp * STEP], f32, tag="sp")
            nc.vector.tensor_copy(
                out=_bc(sp[:], [[rpp * STEP, P], [STEP, rpp], [1, W]]),
                in_=cur[:].rearrange("p (c e) -> p c e", e=W))
            base = r * PBR + int(chunkrow0[i])
            nc.sync.dma_start(
                out=PBUF[base:base + P * rpp, :].rearrange(
                    "(p q) s -> p (q s)", p=P),
                in_=sp[:])
        # rounds: accumulate agg1
        acc = pool.tile([P, COLS * W], f32, bufs=1)
        nc.vector.memset(acc[:], 0.0)
        for r in range(NCORES):
            for si, (a, b) in enumerate(RS):
                ncols = b - a
                ri = pool.tile([P, ncols * P // 16], i16, tag=f"ri{si}")
                nc.sync.dma_start(out=ri[:],
                                  in_=RNDS[si][r * P:(r + 1) * P, :])
                ro = pool.tile([P, ncols * W], f32, tag=f"ro{si}")
                dma_gather_raw(
                    nc.gpsimd,
                    out_ap=ro[:].rearrange("p (c e) -> p c e", e=W),
                    in_ap=PBUF[r * PBR:(r + 1) * PBR, :W],
                    idxs_ap=ri[:], num_idxs=ncols * P, elem_size=W,
                    elem_step=STEP, queue_num=r % NQ)
                nc.vector.tensor_tensor(
                    out=acc[:, a * W:b * W], in0=acc[:, a * W:b * W],
                    in1=ro[:], op=AL.add)
        # epilogue: x = relu((acc@W1)*inv_in + b1); z = (x@W2)*inv_out
        idt = pool.tile([P, COLS], i32, bufs=1)
        nc.sync.dma_start(out=idt[:], in_=ID_[:])
        odt = pool.tile([P, COLS], i32, bufs=1)
        nc.sync.dma_start(out=odt[:], in_=OD_[:])

        def invsqrt(src_t, name):
            f = pool.tile([P, COLS], f32, bufs=1, name=name + "f")
            nc.vector.tensor_copy(out=f[:], in_=src_t[:])
            nc.vector.tensor_scalar(out=f[:], in0=f[:], scalar1=1.0,
                                    scalar2=None, op0=AL.max)
            s = pool.tile([P, COLS], f32, bufs=1, name=name + "s")
            nc.scalar.activation(out=s[:], in_=f[:],
                                 func=mybir.ActivationFunctionType.Sqrt)
            o = pool.tile([P, COLS], f32, bufs=1, name=name + "o")
            nc.vector.reciprocal(out=o[:], in_=s[:])
            return o

        inv_in = invsqrt(idt, "ii")
        inv_out = invsqrt(odt, "io")
        nc.sync.dma_start(out=IID[:], in_=inv_in[:])
        w1 = pool.tile([P, W * W], f32, bufs=1)
        nc.sync.dma_start(out=w1[:], in_=W1R[:])
        w2 = pool.tile([P, W], f32, bufs=1)
        nc.sync.dma_start(out=w2[:], in_=W2R[:])
        b1 = pool.tile([P, W], f32, bufs=1)
        nc.sync.dma_start(out=b1[:], in_=B1R[:])
        t = pool.tile([P, COLS * W], f32, bufs=1)
        tmp = pool.tile([P, COLS * W], f32, bufs=1)
        for f in range(W):
            a_ap = acc[:]
            in0 = _APc(a_ap.tensor, a_ap.offset + f, [[COLS * W, P], [W, COLS], [0, W]])
            w_ap = w1[:]
            in1 = _APc(w_ap.tensor, w_ap.offset + f * W,
                       [[W * W, P], [0, COLS], [1, W]])
            dstt = t if f == 0 else tmp
            nc.vector.tensor_tensor(
                out=dstt[:].rearrange("p (c e) -> p c e", e=W),
                in0=in0, in1=in1, op=AL.mult)
            if f > 0:
                nc.vector.tensor_tensor(out=t[:], in0=t[:], in1=tmp[:], op=AL.add)
        nc.vector.tensor_tensor(
            out=t[:], in0=t[:],
            in1=_bc(inv_in[:], [[COLS, P], [1, COLS], [0, W]]), op=AL.mult)
        nc.vector.tensor_tensor(
            out=t[:], in0=t[:],
            in1=_bc(b1[:], [[W, P], [0, COLS], [1, W]]), op=AL.add)
        x = pool.tile([P, COLS * W], f32, bufs=1)
        nc.vector.tensor_scalar(out=x[:], in0=t[:], scalar1=0.0, scalar2=None,
                                op0=AL.max)
        z = pool.tile([P, COLS], f32, bufs=1)
        ztmp = pool.tile([P, COLS], f32, bufs=1)
        for f in range(W):
            x_ap = x[:]
            in0 = _APc(x_ap.tensor, x_ap.offset + f, [[COLS * W, P], [W, COLS]])
            w_ap = w2[:]
            in1 = _APc(w_ap.tensor, w_ap.offset + f, [[W, P], [0, COLS]])
            dstt = z if f == 0 else ztmp
            nc.vector.tensor_tensor(out=dstt[:], in0=in0, in1=in1, op=AL.mult)
            if f > 0:
                nc.vector.tensor_tensor(out=z[:], in0=z[:], in1=ztmp[:], op=AL.add)
        nc.vector.tensor_tensor(out=z[:], in0=z[:], in1=inv_out[:], op=AL.mult)
        nc.sync.dma_start(out=ZD[:], in_=z[:])
        loop_cm.__exit__(None, None, None)
    nc.finalize()
    return nc


# ---------------------------------------------------------------- launch C
def build_C(plan, reps=0):
    COLS, NTAB, NCH3 = plan["COLS"], plan["NTAB"], plan["NCH3"]
    sched3_r = plan["sched3_r"]
    NCOL3 = NCH3 * CP
    nc = _mk_nc()
    f32, i16, i32 = mybir.dt.float32, mybir.dt.int16, mybir.dt.int32
    ZT = nc.dram_tensor("ZT", [NCORES * NTAB, STEP], f32, kind="ExternalInput")
    GIX = nc.dram_tensor("GIX3", [NCH3 * P, NI // 16], i16, kind="ExternalInput")
    EW3 = nc.dram_tensor("EW3", [NCH3 * P, CP], f32, kind="ExternalInput")
    QI3 = nc.dram_tensor("QI3", [NCH3 * P, CP], f32, kind="ExternalInput")
    HB = nc.dram_tensor("HB", [P, 4], i32, kind="ExternalInput")
    LB = nc.dram_tensor("LB", [P, 4], i32, kind="ExternalInput")
    GH = nc.dram_tensor("GH", [P, 16], f32, kind="ExternalInput")
    CNT = nc.dram_tensor("CNT", [4, 16], i32, kind="ExternalInput")
    B2 = nc.dram_tensor("B2", [4, 1], f32, kind="ExternalInput")
    PO = nc.dram_tensor("PO", [4, 16], f32, kind="ExternalOutput")
    CSD = nc.dram_tensor("CSD", [NCOL3 + 1, 1], f32, kind="Internal")

    with TileContext(nc) as tc, \
         tc.tile_pool(name="sb", bufs=4) as pool, \
         tc.tile_pool(name="ps", bufs=4, space="PSUM") as psp:
        AL = mybir.AluOpType
        ones = pool.tile([P, 1], f32, bufs=1)
        nc.vector.memset(ones[:], 1.0)
        cs = pool.tile([1, NCOL3], f32, bufs=1)
        import contextlib
        loop_cm = tc.For_i(0, reps, 1) if reps else contextlib.nullcontext()
        loop_cm.__enter__()
        for i in range(NCH3):
            r = int(sched3_r[i])
            gi = pool.tile([P, NI // 16], i16, tag="gi")
            nc.sync.dma_start(out=gi[:], in_=GIX[i * P:(i + 1) * P, :])
            zg = pool.tile([P, CP], f32, tag="zg")
            dma_gather_raw(
                nc.gpsimd,
                out_ap=zg[:].rearrange("p (c e) -> p c e", e=1),
                in_ap=ZT[r * NTAB:(r + 1) * NTAB, :1],
                idxs_ap=gi[:], num_idxs=NI, elem_size=1, elem_step=STEP,
                queue_num=i % NQ)
            ew = pool.tile([P, CP], f32, tag="ew")
            nc.sync.dma_start(out=ew[:], in_=EW3[i * P:(i + 1) * P, :])
            qq = pool.tile([P, CP], f32, tag="qq")
            nc.sync.dma_start(out=qq[:], in_=QI3[i * P:(i + 1) * P, :])
            t = pool.tile([P, CP], f32, tag="t")
            nc.vector.tensor_tensor(out=t[:], in0=zg[:], in1=ew[:], op=AL.mult)
            nc.vector.tensor_tensor(out=t[:], in0=t[:], in1=qq[:], op=AL.mult)
            ps = psp.tile([1, CP], f32, tag="cs", bufs=4)
            nc.tensor.matmul(out=ps[:], lhsT=ones[:], rhs=t[:],
                             start=True, stop=True)
            nc.vector.tensor_copy(out=cs[:, i * CP:(i + 1) * CP], in_=ps[:])
        # prefix scan -> CS2 (with leading 0)
        cs2 = pool.tile([1, NCOL3 + 1], f32, bufs=1)
        nc.vector.memset(cs2[:, :1], 0.0)
        zr = pool.tile([1, NCOL3], f32, bufs=1)
        nc.vector.memset(zr[:], 0.0)
        nc.vector.tensor_tensor_scan(
            out=cs2[:, 1:], data0=cs[:], data1=zr[:], initial=0.0,
            op0=AL.add, op1=AL.add)
        csd_ap = _APc(CSD[:].tensor, 0, [[NCOL3 + 1, 1], [1, NCOL3 + 1]])
        wcs = nc.sync.dma_start(out=csd_ap, in_=cs2[:])
        # boundary gathers
        hb = pool.tile([P, 4], i32, bufs=1)
        nc.sync.dma_start(out=hb[:], in_=HB[:])
        lb = pool.tile([P, 4], i32, bufs=1)
        nc.sync.dma_start(out=lb[:], in_=LB[:])
        hi = pool.tile([P, 4], f32, bufs=1)
        lo = pool.tile([P, 4], f32, bufs=1)
        for j in range(4):
            g1 = nc.gpsimd.indirect_dma_start(
                out=hi[:, j:j + 1], out_offset=None, in_=CSD[:],
                in_offset=IndirectOffsetOnAxis(ap=hb[:, j:j + 1], axis=0))
            add_dep_helper(g1.ins, wcs.ins, sync=True, reason="cs before hi")
            g2 = nc.gpsimd.indirect_dma_start(
                out=lo[:, j:j + 1], out_offset=None, in_=CSD[:],
                in_offset=IndirectOffsetOnAxis(ap=lb[:, j:j + 1], axis=0))
            add_dep_helper(g2.ins, wcs.ins, sync=True, reason="cs before lo")
        df = pool.tile([P, 4], f32, bufs=1)
        nc.vector.tensor_tensor(out=df[:], in0=hi[:], in1=lo[:],
                                op=AL.subtract)
        gh = pool.tile([P, 16], f32, bufs=1)
        nc.sync.dma_start(out=gh[:], in_=GH[:])
        pp = psp.tile([4, 16], f32, tag="po", bufs=1)
        nc.tensor.matmul(out=pp[:], lhsT=df[:], rhs=gh[:], start=True, stop=True)
        pl = pool.tile([4, 16], f32, bufs=1)
        nc.vector.tensor_copy(out=pl[:], in_=pp[:])
        cnt = pool.tile([4, 16], i32, bufs=1)
        nc.sync.dma_start(out=cnt[:], in_=CNT[:])
        cf = pool.tile([4, 16], f32, bufs=1)
        nc.vector.tensor_copy(out=cf[:], in_=cnt[:])
        rc = pool.tile([4, 16], f32, bufs=1)
        nc.vector.reciprocal(out=rc[:], in_=cf[:])
        nc.vector.tensor_tensor(out=pl[:], in0=pl[:], in1=rc[:], op=AL.mult)
        b2 = pool.tile([4, 1], f32, bufs=1)
        nc.sync.dma_start(out=b2[:], in_=B2[:])
        nc.vector.tensor_tensor(
            out=pl[:], in0=pl[:], in1=_bc(b2[:], [[1, 4], [0, 16]]), op=AL.add)
        nc.sync.dma_start(out=PO[:], in_=pl[:])
        loop_cm.__exit__(None, None, None)
    nc.finalize()
    return nc


# ---------------------------------------------------------------- entry point
def kernel(atom_features, r, W_emb, b_emb, W1, b1, W2, b2, src, dst,
           graph_ids, num_graphs):
    atom_features = np.asarray(atom_features, FP)
    r = np.asarray(r, FP)
    plan = build_plan(src, dst, graph_ids, r)
    COLS, NTAB, NPAD = plan["COLS"], plan["NTAB"], plan["NPAD"]
    lp, nodecore = plan["lp"], plan["nodecore"]

    # ---- launch A
    ncA = build_A(COLS)
    inA = []
    for c in range(NCORES):
        m = nodecore == c
        ATc = np.zeros((F_IN, NPAD), FP)
        ATc[:, lp[m]] = atom_features[m].T
        inA.append(dict(AT=ATc, WE=np.asarray(W_emb, FP),
                        BE=np.tile(np.asarray(b_emb, FP).reshape(1, W), (P, 1)),
                        OD=plan["outdeg_l"][c]))
    resA = run_bass_kernel_spmd(ncA, inA, core_ids=list(range(NCORES)))
    hsP = np.zeros((NCORES * NTAB, STEP), FP)
    for c in range(NCORES):
        hsd = resA.results[c]["HS"].reshape(P, COLS, W)
        hsP[c * NTAB:c * NTAB + NPAD, :W] = (
            hsd.transpose(1, 0, 2).reshape(NPAD, W))

    # ---- launch B
    ncB = build_B(plan)
    inB = []
    for c in range(NCORES):
        d = dict(HSP=hsP, GIX=plan["gidx2"][c], RP=plan["rP2"][c],
                 IDG=plan["indeg_l"][c], ODG=plan["outdeg_l"][c],
                 W1R=np.tile(np.asarray(W1, FP).reshape(1, W * W), (P, 1)),
                 W2R=np.tile(np.asarray(W2, FP).reshape(1, W), (P, 1)),
                 B1R=np.tile(np.asarray(b1, FP).reshape(1, W), (P, 1)))
        for si in range(len(plan["RS"])):
            d[f"RND{si}"] = plan["rnd"][si][c]
        inB.append(d)
    resB = run_bass_kernel_spmd(ncB, inB, core_ids=list(range(NCORES)))

    # ---- glue: z table, ew3/qin3 streams
    NCH2, NCH3 = plan["NCH2"], plan["NCH3"]
    ztab = np.zeros((NCORES * NTAB, STEP), FP)
    ew3 = np.zeros((NCORES, NCH3 * P, CP), FP)
    qin3 = np.zeros((NCORES, NCH3 * P, CP), FP)
    src_a = np.asarray(src, np.int64)
    dst_a = np.asarray(dst, np.int64)
    for c in range(NCORES):
        zd = resB.results[c]["ZD"].reshape(P, COLS)
        ztab[c * NTAB:c * NTAB + NPAD, 0] = zd.T.reshape(NPAD)
        ewflat = (resB.results[c]["EWD"].reshape(NCH2, P, CP)
                  .transpose(0, 2, 1).reshape(-1))
        invin_flat = resB.results[c]["IID"].reshape(P, COLS).T.reshape(NPAD)
        em = np.nonzero(plan["ecore"] == c)[0]
        s3 = plan["l3slot"][em]
        s2 = plan["l2slot"][em]
        f3 = np.zeros(NCH3 * NI, FP)
        f3[s3] = ewflat[s2]
        ew3[c] = f3.reshape(NCH3, CP, P).transpose(0, 2, 1).reshape(NCH3 * P, CP)
        q3 = np.zeros(NCH3 * NI, FP)
        q3[s3] = invin_flat[lp[dst_a[em]]]
        qin3[c] = q3.reshape(NCH3, CP, P).transpose(0, 2, 1).reshape(NCH3 * P, CP)

    # ---- launch C
    ncC = build_C(plan)
    inC = []
    for c in range(NCORES):
        inC.append(dict(ZT=ztab, GIX3=plan["gidx3"][c], EW3=ew3[c],
                        QI3=qin3[c],
                        HB=plan["hibnd"][c].reshape(4, P).T.copy().astype(np.int32),
                        LB=plan["lobnd"][c].reshape(4, P).T.copy().astype(np.int32),
                        GH=plan["grouphot"], CNT=plan["cnt_pg"][c],
                        B2=np.full((4, 1), np.asarray(b2, FP).reshape(-1)[0], FP)))
    resC = run_bass_kernel_spmd(ncC, inC, core_ids=list(range(NCORES)))

    out = np.zeros(N_GRAPHS, FP)
    for c in range(NCORES):
        po = resC.results[c]["PO"]
        for gl_ in range(GPC):
            out[c * GPC + gl_] = po[gl_ // 16, gl_ % 16]
    return out



# revision 2
# speedup vs baseline: 2.4143x; 2.4143x over previous
"""GCNSimple v2: 8-core data-parallel, 2 launches.

A: embed h=(atom@WE+b)*io -> bf16 table window; ew=exp(-||r||^2) for all
   edge slots (tier1 + overflow layout); iid=rsqrt(indeg) out.
B: tier1 fixed-4 (node,window) gather cells -> ew-mult -> 2-level tree ->
   SBUF accumulate; overflow edges gather + scatter-add into DRAM ACC;
   epilogue x=relu((agg@W1)*ii+b1), z=(x@W2)*io; dense pooling matmul
   PART[512] = sum_s z_s * K'[s,:] (K' built on host from device ew/iid).
Host: index/layout preprocessing, K' bincount, final 8-way partial sum.
"""
import sys
sys.path.insert(0, "/opt/trn_rl_repo")
import numpy as np
import ml_dtypes

import concourse.bacc as bacc
import concourse.mybir as mybir
from concourse import ap_utils
from concourse.bass import MemorySpace, AP as _APc, IndirectOffsetOnAxis
from concourse.tile import TileContext, add_dep_helper
from concourse._compat import exact_div
from concourse.bass_utils import run_bass_kernel_spmd

P = 128
NCORES = 8
N_NODES = 200_000
N_EDGES = 6_400_000
N_GRAPHS = 512
GPC = N_GRAPHS // NCORES
F_IN = 92
W = 10
K1 = 4                  # tier1 slots per (node, window)
CPT = 64                # slot-cols per tier1 gather chunk
OVC = 8192              # target overflow idx per chunk
NQ = 4
FP = np.float32
BF = ml_dtypes.bfloat16
ROWE = 128              # table row elems (bf16) = 256B


# ---------------------------------------------------------------- raw gather
def dma_gather_raw(gp, out_ap, in_ap, idxs_ap, num_idxs, elem_size, elem_step,
                   queue_num=0):
    gp._assert_queue_num(queue_num)
    assert idxs_ap.dtype == mybir.dt.int16
    assert in_ap.dtype == out_ap.dtype
    assert in_ap.space == MemorySpace.DRAM
    assert ap_utils.ap_is_contiguous(in_ap.ap[1:])
    assert ap_utils.ap_is_contiguous(out_ap.ap[1:])
    assert ap_utils.ap_is_contiguous(idxs_ap.ap[1:])
    assert in_ap.ap[-1][1] == out_ap.ap[-1][1] == elem_size
    assert out_ap.ap[0][1] * out_ap.ap[1][1] == (num_idxs + 127) // 128 * 128
    assert in_ap.ap[0][0] == elem_step
    stride_bytes_256 = exact_div(elem_step * mybir.dt.size(in_ap.dtype), 256)
    return gp.add_instruction(
        mybir.InstDMAGatherAnt(
            name=gp.bass.get_next_instruction_name(),
            ins=[*gp.lower_ap_dma(in_ap, for_custom_bir_dma=True),
                 gp.lower_ap(idxs_ap),
                 gp.lower_val_access(gp.to_reg(num_idxs))],
            outs=[gp.lower_ap(out_ap)],
            transpose=False, num_idxs=num_idxs, elem_size=elem_size,
            stride_bytes_256=stride_bytes_256, gen_mode=0,
            single_packet=False, queue_num=queue_num,
            sbuf_tokens_per_rank=0, sbuf_free_dim_per_rank=0,
            sbuf_free_dim_pad_per_rank=0, sbuf_byte_offset=0,
        ))


def wrap_idx16(logical_idx, num_idxs):
    w = np.zeros((16, num_idxs // 16), np.int16)
    ar = np.arange(num_idxs)
    w[ar % 16, ar // 16] = logical_idx.astype(np.int16)
    return np.tile(w, (8, 1))


def _bc(t_ap, dims):
    return _APc(t_ap.tensor, t_ap.offset, [list(d) for d in dims])


# ---------------------------------------------------------------- host plan
def build_plan2(src, dst, graph_ids, r_in):
    src = np.asarray(src, np.int64)
    dst = np.asarray(dst, np.int64)
    gid = np.asarray(graph_ids, np.int64)
    r_in = np.asarray(r_in, FP)

    corenode = gid // GPC
    first = np.searchsorted(corenode, np.arange(NCORES + 1))
    cnt_core = np.diff(first)
    NPADU = int((cnt_core.max() + P - 1) // P * P)
    COLS = NPADU // P
    NTAB = NPADU + 1
    assert NTAB < 32767
    slot = np.arange(N_NODES) - first[corenode]

    outdeg = np.bincount(src, minlength=N_NODES).astype(np.int32)
    indeg = np.bincount(dst, minlength=N_NODES).astype(np.int32)

    ecore = corenode[dst]
    ewin = corenode[src]
    WCOLS = COLS * K1               # tier1 cols per window region
    T1COLS = NCORES * WCOLS         # tier1 cols per core

    # ---- per-core tier1 fill + overflow edge lists (sorted by w, dst)
    pre = []
    ovcnt = np.zeros((NCORES, NCORES), np.int64)   # [core, window] -> edges
    for c in range(NCORES):
        em = np.nonzero(ecore == c)[0]
        ew_, ed_ = ewin[em], dst[em]
        order = np.lexsort((ed_, ew_))
        em, ew_, ed_ = em[order], ew_[order], ed_[order]
        key = ew_ * N_NODES + ed_
        chg = np.empty(len(em), bool)
        chg[0] = True
        chg[1:] = key[1:] != key[:-1]
        gstart = np.where(chg, np.arange(len(em)), 0)
        np.maximum.accumulate(gstart, out=gstart)
        j = np.arange(len(em)) - gstart
        t1m = j < K1
        ovm = ~t1m
        ovcnt[c] = np.bincount(ew_[ovm], minlength=NCORES)
        pre.append(dict(em=em, ew=ew_, ed=ed_, j=j, t1m=t1m, ovm=ovm))

    # ---- uniform overflow schedule: per window, chunk layout shared by all
    # cores; each chunk is (window, ncols) with idx padded by -1.
    ovsched = []                      # list of (window, npad_cols)
    for w_ in range(NCORES):
        n = int(ovcnt[:, w_].max())
        done = 0
        while done < n:
            take = min(OVC, n - done)
            ovsched.append((w_, (take + P - 1) // P))
            done += take
    OVTOT = sum(nc_ for _, nc_ in ovsched)
    RCOLS = T1COLS + OVTOT
    ovbase = np.zeros(len(ovsched), np.int64)     # col base within ov region
    b = 0
    for i, (w_, nc_) in enumerate(ovsched):
        ovbase[i] = b
        b += nc_
    # per-window chunk id list and base edge offsets (uniform)
    wchunks = {w_: [i for i, (ww, _) in enumerate(ovsched) if ww == w_]
               for w_ in range(NCORES)}

    # ---- per-core streams
    cores = []
    epos_all = np.full(N_EDGES, -1, np.int64)   # position in core's stream
    for c in range(NCORES):
        pc = pre[c]
        em, ew_, ed_, t1m, ovm, j = (pc["em"], pc["ew"], pc["ed"], pc["t1m"],
                                     pc["ovm"], pc["j"])
        # tier1
        sd = slot[ed_[t1m]]
        pos1 = (ew_[t1m] * WCOLS + (sd // P) * K1 + j[t1m]) * P + (sd % P)
        gl = np.full(RCOLS * P, NTAB - 1, np.int32)
        rr = np.zeros((RCOLS * P, 3), FP)
        rr[:, 0] = 100.0
        gl[pos1] = slot[src[em[t1m]]]
        rr[pos1] = r_in[em[t1m]]
        epos_all[em[t1m]] = pos1
        # overflow: fill per window into the uniform chunks
        sx = np.full(RCOLS * P, NPADU, np.int32)   # trash row pad
        emo, ewo_, edo_ = em[ovm], ew_[ovm], ed_[ovm]
        for w_ in range(NCORES):
            sel = ewo_ == w_
            eids = emo[sel]
            dsts = edo_[sel]
            n = len(eids)
            # positions: fill chunks of this window in order
            off = 0
            for ci in wchunks[w_]:
                cap = ovsched[ci][1] * P
                if ci == wchunks[w_][-1]:
                    cap_real = cap
                else:
                    cap_real = min(cap, OVC)
                take = min(n - off, cap_real)
                if take <= 0:
                    break
                base = (T1COLS + ovbase[ci]) * P
                posi = base + np.arange(take)
                ee = eids[off:off + take]
                gl[posi] = slot[src[ee]]
                sx[posi] = slot[dsts[off:off + take]]
                rr[posi] = r_in[ee]
                epos_all[ee] = posi
                off += take
            assert off == n
        cores.append(dict(gl=gl, sx=sx, rr=rr))

    # tier1 chunk col spans within each window region
    t1spans = []
    a = 0
    while a < WCOLS:
        t1spans.append((a, min(a + CPT, WCOLS)))
        a = min(a + CPT, WCOLS)

    return dict(NPADU=NPADU, COLS=COLS, NTAB=NTAB, WCOLS=WCOLS,
                T1COLS=T1COLS, RCOLS=RCOLS, OVTOT=OVTOT,
                t1spans=t1spans, ovsched=ovsched, ovbase=ovbase,
                cores=cores, slot=slot, corenode=corenode, first=first,
                outdeg=outdeg, indeg=indeg, epos_all=epos_all,
                src=src, dst=dst, gid=gid, cnt_core=cnt_core)


def pack_core_inputs(plan):
    """Per-core GIX/SIX (wrapped idx) and R1 streams."""
    RCOLS, T1COLS = plan["RCOLS"], plan["T1COLS"]
    packs = []
    for c in range(NCORES):
        cc = plan["cores"][c]
        gixw = wrap_idx16(cc["gl"], RCOLS * P)        # [128, RCOLS*8]
        sixw = wrap_idx16(cc["sx"][T1COLS * P:], plan["OVTOT"] * P)
        r1 = (cc["rr"].reshape(RCOLS, P, 3).transpose(1, 0, 2)
              .reshape(P, RCOLS * 3))
        packs.append(dict(GIX=gixw, SIX=sixw, R1=r1))
    return packs


def nodearr(plan, vals, c):
    COLS = plan["COLS"]
    out = np.zeros((P, COLS), vals.dtype)
    m = plan["corenode"] == c
    s = plan["slot"][m]
    out[s % P, s // P] = vals[m]
    return out
